# revision 1
# baseline (speedup 1.0000x reference)
"""Trainium2 Bass kernel for NormalAttention (embedded gaussian, non-local block).

Reference computation per batch sample b (B=8, C=256, Ck=64, N=48*48=2304):
    q = Wq @ x + bq            (64, N)
    k = Wk @ x + bk            (64, N)
    e[i,j] = q[:,i] . k[:,j]   (N, N)
    E = exp(e);  E[i,j] /= sum_j E[i,j]
    v = Wv @ x + bv            (256, N)
    att[c,j] = sum_i v[c,i] * E[i,j]
    out = Wg @ att + bg        (256, N)

Sharding: pure data parallel, one batch sample per NeuronCore (8 cores).

Per-core kernel structure (cost-model timed at ~82.0us/core):
  - all matmuls in bf16 (1 cycle/row on PE vs 4 for fp32); rel err ~4.5e-3.
  - energy rows computed one 128-row i-chunk at a time into ping-pong PSUM
    tiles ((128,1280)+(128,1024) = 5 banks); exp on ACT engine straight out
    of PSUM into resident bf16 expA/expB SBUF tensors. Steady-state phase 1
    is ACT-paced at ~2.48us/chunk (exp 1.92us + per-op init + accum read).
  - row sums: ACT accum_out on the 1280-half + DVE tensor_reduce on the
    1024-half (expE split into two tiles so the reduce never falsely
    serializes against the next exp's SBUF write); row normalization is
    folded into V^T (vt[i,:] *= 1/s[i]) on DVE.
  - the gamma 1x1 conv is folded into the V projection on the host
    (W_comb = (Wg@Wv)^T, bvg = Wg@bv), so pass 2 (out = vg^T.T @ expE)
    directly produces final outputs: PSUM-accumulate over the 18 i-chunks,
    add gamma_bias on DVE, store. 3 of the 10 (oc, j-tile) accumulator
    groups stream into phase 1's idle PE slots (PSUM-bank limited); the
    remaining groups run in phase 2, the first two out of the freed energy
    banks, smallest j-tiles last to minimize the tail.
  - head: PE warmup matmuls + a 1-element exp (prefetches the ACT
    function table) under the input DMAs; x arrives in 3 column pieces
    ordered to unblock the Q/K projection chain (k-bias on DVE and q-bias
    on ACT run as parallel PSUM->SBUF chains).
"""

import os
import sys

sys.path.insert(0, "/opt/trn_rl_repo")

# NTFF tracing is unavailable through this container's axon client; make sure
# a stray BASS_TRACE in the environment can't break the execution path.
os.environ["BASS_NEVER_TRACE"] = "1"

# This kernel executes through the axon-proxied PJRT backend. If the caller's
# environment pinned jax to CPU (common for reference-side runs), drop the pin
# before jax initializes so the TRN2 devices stay discoverable.
_jp = os.environ.get("JAX_PLATFORMS")
if _jp and "axon" not in _jp and "jax" not in sys.modules:
    os.environ.pop("JAX_PLATFORMS", None)

import numpy as np
import ml_dtypes

import concourse.bass as bass
import concourse.mybir as mybir
import concourse.tile as tile
from concourse import bacc
from concourse.bass_utils import run_bass_kernel_spmd

B, C, CK, H, W = 8, 256, 64, 48, 48
N = H * W            # 2304
P = 128
NI = N // P          # 18 i-chunks
NCORES = 8

BF16 = mybir.dt.bfloat16
F32 = mybir.dt.float32
AF = mybir.ActivationFunctionType
ALU = mybir.AluOpType
AX = mybir.AxisListType

# energy ping-pong PSUM split: (128,1280)=3 banks + (128,1024)=2 banks.
# expE is likewise stored as two SBUF tiles (A: j<1280, B: j>=1280) so the
# DVE row-sum reduce of half B never falsely serializes with the next exp.
EA, EB = 1280, 1024
E_SPLITS = [
    (0, EA, "engA", [(0, 512), (512, 512), (1024, 256)]),
    (EA, EB, "engB", [(0, 512), (512, 512)]),
]
# pass-2 j-tiling must nest inside the A/B split
J_TILES = [(0, 512), (512, 512), (1024, 256), (1280, 512), (1792, 512)]
# pass-2 groups (ch, j0), jw=512, streamed into phase 1 (all within half A)
STREAM_GROUPS = [(0, 0), (1, 0), (0, 512)]

N_WARM = 5           # PE warmup matmuls issued under the input DMAs


def _build_nc():
    nc = bacc.Bacc("TRN2", target_bir_lowering=False, debug=False,
                   num_devices=NCORES)

    x_d = nc.dram_tensor("x", [2, P, N], BF16, kind="ExternalInput")
    wqk_d = nc.dram_tensor("wqk", [P, 2 * P], BF16, kind="ExternalInput")
    wrest_d = nc.dram_tensor("wrest", [P, 2 * C], BF16, kind="ExternalInput")
    fblob_d = nc.dram_tensor("fblob", [P, C + 4], F32, kind="ExternalInput")
    out_d = nc.dram_tensor("out", [2, P, N], F32, kind="ExternalOutput")
    warm_d = nc.dram_tensor("warm", [P, 1], F32, kind="ExternalOutput")

    with tile.TileContext(nc) as tc:
        with (
            tc.tile_pool(name="consts", bufs=1) as consts,
            tc.tile_pool(name="big", bufs=1) as big,
            tc.tile_pool(name="work", bufs=6) as work,
            tc.tile_pool(name="ps_big", bufs=1, space="PSUM") as ps_big,
            tc.tile_pool(name="ps_sm", bufs=2, space="PSUM") as ps_sm,
            tc.tile_pool(name="ps_st", bufs=1, space="PSUM") as ps_st,
        ):
            # ---------------- PE warmup under the input DMAs ----------------
            dummy = consts.tile([P, 512], BF16)
            nc.gpsimd.memset(dummy[:], 0)
            warm_sb = consts.tile([P, 1], F32)
            # 1-element exp: forces the implicit ACT_TABLE_LOAD (~1.3us) to
            # run at t~0 under the DMAs instead of gating the first q-bias
            nc.scalar.activation(warm_sb[0:1, 0:1], dummy[0:1, 0:1], AF.Exp)
            psd = ps_sm.tile([P, 512], F32, tag="sm")
            for w in range(N_WARM):
                nc.tensor.matmul(psd[:], dummy[:, :P], dummy[:],
                                 start=(w == 0), stop=(w == N_WARM - 1))
            nc.vector.tensor_copy(warm_sb, psd[:, 0:1])
            nc.sync.dma_start(warm_d[:], warm_sb)

            # ---------------- inputs ----------------
            # order: biases + wqk (tiny) first, then x in j-halves so the
            # first Q/K projection tiles can start before x fully lands
            xt = big.tile([P, 2, N], BF16)
            fblob = consts.tile([P, C + 4], F32)
            wqk = consts.tile([P, 2 * P], BF16)
            x_r = x_d[:].rearrange("c p n -> p c n")
            nc.sync.dma_start(xt[:, :, 0:512], x_r[:, :, 0:512])
            nc.sync.dma_start(fblob, fblob_d[:])
            nc.sync.dma_start(wqk[:], wqk_d[:])
            nc.sync.dma_start(xt[:, :, 512:1280], x_r[:, :, 512:1280])
            nc.sync.dma_start(xt[:, :, 1280:N], x_r[:, :, 1280:N])
            wrest = consts.tile([P, 2 * C], BF16)
            nc.sync.dma_start(wrest[:], wrest_d[:])

            # combined projection weight W_comb = (Wg @ Wv)^T: the gamma
            # 1x1 conv is folded into the V projection on the host, so
            # pass-2 outputs are final (no gamma matmuls, no att copies)
            def wv(c):
                return wrest[:, c * C:(c + 1) * C]

            qb = fblob[0:CK, 0:1]
            kb = fblob[0:CK, 1:2]
            vb = fblob[:, 4:C + 4]
            gbias = fblob[:, 2:4]

            # ---------------- Q / K projections ----------------
            q_t = big.tile([CK, N], BF16)
            k_t = big.tile([CK, N], BF16)

            # k-bias lands on DVE, q-bias on ACT (parallel PSUM->SBUF chains;
            # the energy matmuls are gated mostly on k_t). The q-bias for a
            # j-tile can be deferred (only q_t[:, k*128:(k+1)*128] gates
            # chunk k's energy row).
            def psk_mms(j0, jw, pool=None, tag="sm", on_act=False):
                psk = (pool or ps_sm).tile([P, 512], F32, tag=tag, name="psk")
                for c in range(2):
                    nc.tensor.matmul(psk[:CK, :jw],
                                     wqk[:, c * P + CK:(c + 1) * P],
                                     xt[:, c, j0:j0 + jw],
                                     start=(c == 0), stop=(c == 1))
                if on_act:
                    nc.scalar.activation(k_t[:, j0:j0 + jw], psk[:CK, :jw],
                                         AF.Identity, bias=kb)
                else:
                    nc.vector.tensor_scalar_add(k_t[:, j0:j0 + jw],
                                                psk[:CK, :jw], kb)

            def psq_mms(j0, jw, on_act=False):
                psq = ps_sm.tile([P, 512], F32, tag="sm", name="psq")
                for c in range(2):
                    nc.tensor.matmul(psq[:CK, :jw], wqk[:, c * P:c * P + CK],
                                     xt[:, c, j0:j0 + jw],
                                     start=(c == 0), stop=(c == 1))
                if on_act:
                    # ACT is idle during the head; q_t[:, :128] gates exp(0)
                    nc.scalar.activation(q_t[:, j0:j0 + jw], psq[:CK, :jw],
                                         AF.Identity, bias=qb)
                else:
                    nc.vector.tensor_scalar_add(q_t[:, j0:j0 + jw],
                                                psq[:CK, :jw], qb)

            # shared big SBUF tensors
            vt = big.tile([P, NI, C], BF16)       # V^T, later scaled by 1/s
            expA = big.tile([P, NI, EA], BF16)
            expB = big.tile([P, NI, EB], BF16)
            s_half = big.tile([P, NI, 2], F32)
            invs = big.tile([P, NI], F32)

            eps_of = {}

            def emit_energy(kk, part):
                (base, width, tag, subs) = E_SPLITS[part]
                eps = ps_big.tile([P, width], F32, tag=tag, name=f"eps{part}")
                for (o0, ow) in subs:
                    nc.tensor.matmul(
                        eps[:, o0:o0 + ow],
                        q_t[:, kk * P:(kk + 1) * P],
                        k_t[:, base + o0:base + o0 + ow],
                        start=True, stop=True)
                eps_of.setdefault(kk, [None, None])[part] = eps

            # Q/K for j < 1280, then E(0) half A right away; rest of Q/K,
            # then E(0) half B -- gets the first exp started ASAP. The
            # q-biases of the last two j-tiles are deferred into the loop
            # (not needed until energy chunk 10) to keep ACT clear.
            # critical chain to exp(0): k_t[:, 0:1280] + q_t[:, 0:128];
            # run both bias chains (DVE for k, ACT for q) concurrently,
            # borrowing the stream slot (idle until chunk 1) for psk j1
            psk_mms(*J_TILES[0])
            psq_mms(*J_TILES[0], on_act=True)
            psk_mms(*J_TILES[1], pool=ps_st, tag="st", on_act=True)
            psk_mms(*J_TILES[2])
            emit_energy(0, 0)
            psq_mms(*J_TILES[1])
            psq_mms(*J_TILES[2])
            for (j0, jw) in J_TILES[3:]:
                psk_mms(j0, jw)
            emit_energy(0, 1)
            psq_mms(*J_TILES[3])
            psq_mms(*J_TILES[4])

            # ---------------- V^T projection ----------------
            # emitted after E(0); demoted in scheduler priority so the
            # chains only fill genuine PE idle slots instead of front-
            # running the energy matmuls of the first pass-1 chunks
            with tc.high_priority(offset=-100000):
                for i in range(NI):
                    psv = ps_sm.tile([P, 512], F32, tag="sm")
                    for c in range(2):
                        nc.tensor.matmul(psv[:, :C],
                                         xt[:, c, i * P:(i + 1) * P],
                                         wv(c), start=(c == 0), stop=(c == 1))
                    nc.vector.tensor_tensor(vt[:, i], psv[:, :C], vb, ALU.add)

            # ---------------- pass 1 pipeline ----------------
            st_tiles = []
            for k in range(NI):
                # exp of both energy halves; row-sum of half A via ACT accum
                nc.scalar.activation(
                    out=expA[:, k, :], in_=eps_of[k][0][:],
                    func=AF.Exp, accum_out=s_half[:, k, 0:1])
                nc.scalar.activation(
                    out=expB[:, k, :], in_=eps_of[k][1][:],
                    func=AF.Exp)
                # row-sum of half B on DVE; s = sA + sB; invs = 1/s
                nc.vector.tensor_reduce(
                    s_half[:, k, 1:2], expB[:, k, :],
                    axis=AX.X, op=ALU.add)
                nc.vector.tensor_tensor(invs[:, k:k + 1], s_half[:, k, 0:1],
                                        s_half[:, k, 1:2], ALU.add)
                nc.vector.reciprocal(invs[:, k:k + 1], invs[:, k:k + 1])
                nc.vector.tensor_scalar_mul(vt[:, k], vt[:, k],
                                            invs[:, k:k + 1])
                # PE order per chunk: E(k+1)A (unblocks the next exp ASAP),
                # then streamed pass-2 matmuls for chunk k-1 covering the
                # wait for exp(k)B's PSUM read, then E(k+1)B.
                if k + 1 < NI:
                    emit_energy(k + 1, 0)
                if k >= 1:
                    kk = k - 1
                    if kk == 0:
                        st_tiles = [
                            (ps_st if gi == 0 else ps_sm).tile(
                                [P, 512], F32,
                                tag=("st" if gi == 0 else "sm"),
                                name=f"stream_{gi}")
                            for gi in range(len(STREAM_GROUPS))
                        ]
                    for gi, (ch, j0) in enumerate(STREAM_GROUPS):
                        nc.tensor.matmul(
                            st_tiles[gi][:],
                            vt[:, kk, ch * P:(ch + 1) * P],
                            expA[:, kk, j0:j0 + 512],
                            start=(kk == 0), stop=False)
                if k + 1 < NI:
                    emit_energy(k + 1, 1)

            # ---------------- pass 2 ----------------
            def exp_slice(i, j0, jw):
                if j0 + jw <= EA:
                    return expA[:, i, j0:j0 + jw]
                return expB[:, i, j0 - EA:j0 - EA + jw]

            def emit_out(oc, j0, jw, psum_ap):
                ot = work.tile([P, 512], F32, tag="out")
                nc.vector.tensor_scalar_add(ot[:, :jw], psum_ap,
                                            gbias[:, oc:oc + 1])
                nc.sync.dma_start(out_d[oc, :, j0:j0 + jw], ot[:, :jw])

            def full_group(oc, j0, jw, pool=None, tag="sm"):
                aps = (pool or ps_sm).tile([P, 512], F32, tag=tag, name="aps")
                for i in range(NI):
                    nc.tensor.matmul(
                        aps[:, :jw],
                        vt[:, i, oc * P:(oc + 1) * P],
                        exp_slice(i, j0, jw),
                        start=(i == 0), stop=(i == NI - 1))
                emit_out(oc, j0, jw, aps[:, :jw])

            # First two groups run out of the (now free) energy-PSUM banks
            # so PE never waits for the stream slots to clear; the stream
            # groups close out and store immediately. Smallest tiles last.
            full_group(1, 512, 512, pool=ps_big, tag="engA")
            full_group(0, 1280, 512, pool=ps_big, tag="engB")

            for gi, (oc, j0) in enumerate(STREAM_GROUPS):
                nc.tensor.matmul(
                    st_tiles[gi][:],
                    vt[:, NI - 1, oc * P:(oc + 1) * P],
                    expA[:, NI - 1, j0:j0 + 512],
                    start=False, stop=True)
                emit_out(oc, j0, 512, st_tiles[gi][:])

            full_group(1, 1280, 512)
            full_group(0, 1792, 512)
            full_group(1, 1792, 512, pool=ps_st, tag="st")
            full_group(0, 1024, 256, pool=ps_big, tag="engA")
            full_group(1, 1024, 256, pool=ps_big, tag="engB")

    nc.compile()
    return nc


_NC_CACHE = []


def _get_nc():
    if not _NC_CACHE:
        _NC_CACHE.append(_build_nc())
    return _NC_CACHE[0]


def _prep_inputs(x, query_weight, query_bias, key_weight, key_bias,
                 value_weight, value_bias, gamma_weight, gamma_bias):
    bf16 = ml_dtypes.bfloat16
    x = np.asarray(x, np.float32).reshape(B, C, N)
    qw = np.asarray(query_weight, np.float32)[:, :, 0, 0]   # (64, 256)
    kw = np.asarray(key_weight, np.float32)[:, :, 0, 0]     # (64, 256)
    vw = np.asarray(value_weight, np.float32)[:, :, 0, 0]   # (256, 256)
    gw = np.asarray(gamma_weight, np.float32)[:, :, 0, 0]   # (256, 256)

    # wqk[p, c*128+m] = W_cat^T[c*128+p, m]  (W_cat = [Wq; Wk], (128, 256))
    wcat_t = np.concatenate([qw, kw], axis=0).T              # (256, 128)
    wqk = np.ascontiguousarray(
        wcat_t.reshape(2, P, P).transpose(1, 0, 2).reshape(P, 2 * P))

    # the gamma 1x1 conv folds into the V projection:
    #   out = Wg @ (VS^T E) + bg = ((Wv^T Wg^T-projected X)^T-scaled E) + bg
    # so the device projects x with W_comb = (Wg @ Wv)^T and the value bias
    # becomes bvg = Wg @ bv.
    w_comb = (gw @ vw).T                                    # (c_in, o)
    wrest = np.ascontiguousarray(
        w_comb.reshape(2, P, C).transpose(1, 0, 2).reshape(P, 2 * C))
    bvg = gw @ np.asarray(value_bias, np.float32)

    fblob = np.zeros((P, C + 4), np.float32)
    fblob[0:CK, 0] = np.asarray(query_bias, np.float32)
    fblob[0:CK, 1] = np.asarray(key_bias, np.float32)
    fblob[:, 2:4] = np.asarray(gamma_bias, np.float32).reshape(2, P).T
    fblob[:, 4:C + 4] = bvg[None, :]

    base = {
        "wqk": wqk.astype(bf16),
        "wrest": wrest.astype(bf16),
        "fblob": fblob,
    }
    in_maps = []
    for b in range(B):
        m = dict(base)
        m["x"] = x[b].reshape(2, P, N).astype(bf16)
        in_maps.append(m)
    return in_maps


def kernel(x, query_weight, query_bias, key_weight, key_bias,
           value_weight, value_bias, gamma_weight, gamma_bias, k):
    assert int(k) == C // CK
    in_maps = _prep_inputs(x, query_weight, query_bias, key_weight, key_bias,
                           value_weight, value_bias, gamma_weight, gamma_bias)
    nc = _get_nc()
    res = run_bass_kernel_spmd(nc, in_maps, core_ids=list(range(NCORES)))

    out = np.empty((B, C, H, W), np.float32)
    for b in range(B):
        out[b] = res.results[b]["out"].reshape(C, H, W)
    return out



# revision 39
# speedup vs baseline: 1.2308x; 1.2308x over previous
"""Trainium2 Bass kernel for NormalAttention (embedded gaussian, non-local block).

Reference computation per batch sample b (B=8, C=256, Ck=64, N=48*48=2304):
    q = Wq @ x + bq            (64, N)
    k = Wk @ x + bk            (64, N)
    e[i,j] = q[:,i] . k[:,j]   (N, N)
    E = exp(e);  E[i,j] /= sum_j E[i,j]
    v = Wv @ x + bv            (256, N)
    att[c,j] = sum_i v[c,i] * E[i,j]
    out = Wg @ att + bg        (256, N)

Sharding: pure data parallel, one batch sample per NeuronCore (8 cores).

Kernel structure (v2; cost-model timed):
  - phase 1 is an ACT-paced exp pipeline over 18 query chunks of 128 rows
    (pace ~2.29us/chunk = the exp work itself + per-instruction access
    bubbles); energy rows land in ping-pong PSUM halves A(1280)+B(1024);
    exp writes E directly as fp8e4 into per-pair SBUF tiles e8[pair][2][N]
    (consecutive chunks pair up for DoubleRow pass-2 matmuls).
  - row sums: DVE tensor_reduce takes the A half directly from fp8;
    Pool pre-folds the B half; normalization (x 4096/s) is folded into V^T
    (vts bf16, v8 fp8 via Pool cast); the global 1/4096 is removed in the
    output bias step.
  - gamma 1x1 conv folded into the V projection on the host (W_comb =
    (Wg@Wv)^T, bvg = Wg@bv), so pass 2 directly produces final outputs.
  - pass 2: 3 streamed groups (bf16 vts x fp8 E) accumulate during phase 1
    in PE idle; the remaining 7 (oc, j-tile) groups run as fp8 DoubleRow
    pair matmuls after the last exp, all concurrently resident in PSUM
    (sub-sliced banks), biases alternate ACT/DVE, outputs staged bf16 and
    stored in 4 consolidated DMAs.
"""

import os
import sys

sys.path.insert(0, "/opt/trn_rl_repo")

os.environ["BASS_NEVER_TRACE"] = "1"

_jp = os.environ.get("JAX_PLATFORMS")
if _jp and "axon" not in _jp and "jax" not in sys.modules:
    os.environ.pop("JAX_PLATFORMS", None)

import numpy as np
import ml_dtypes

import concourse.bass as bass
import concourse.mybir as mybir
import concourse.tile as tile
from concourse import bacc
from concourse.bass_utils import run_bass_kernel_spmd

B, C, CK, H, W = 8, 256, 64, 48, 48
N = H * W            # 2304
P = 128
NI = N // P          # 18 i-chunks
NP = NI // 2         # 9 chunk pairs (DoubleRow)
NCORES = 8
Z = 4096.0           # global scale folded into vts/v8, removed at output

BF16 = mybir.dt.bfloat16
F8 = mybir.dt.float8e4
F32 = mybir.dt.float32
AF = mybir.ActivationFunctionType
ALU = mybir.AluOpType
AX = mybir.AxisListType
DR = mybir.MatmulPerfMode.DoubleRow

# energy ping-pong PSUM split: (128,1280)=3 banks + (128,1024)=2 banks.
EA, EB = 1280, 1024
A_SUBS = [(0, 512), (512, 512), (1024, 256)]
B_SUBS = [(0, 512), (512, 512)]

# pass-2 groups (oc, j0, jw): 2 oc x (4x512 + 256) j-tiles
STREAM_GROUPS = [(0, 0, 512), (1, 0, 512)]
# tail groups hosted concurrently in sub-sliced PSUM buffers:
#   pA (1280 f32): 512 + 512 + 256 ; pB (1024): 512 + 512 ;
#   pU: 512 ; pS buf1: 512 ; pS buf2: 256
TAIL_A = [(0, 512, 512, 0), (1, 512, 512, 512), (0, 2048, 256, 1024)]
TAIL_B = [(0, 1024, 512, 0), (1, 1024, 512, 512)]
TAIL_U = [(0, 1536, 512)]
TAIL_S = [(1, 1536, 512), (1, 2048, 256)]

N_WARM = 7


def _build_nc():
    nc = bacc.Bacc("TRN2", target_bir_lowering=False, debug=False,
                   num_devices=NCORES)

    x_d = nc.dram_tensor("x", [2, P, N], BF16, kind="ExternalInput")
    wqk_d = nc.dram_tensor("wqk", [P, 2 * P], BF16, kind="ExternalInput")
    wrest_d = nc.dram_tensor("wrest", [P, 2 * C], BF16, kind="ExternalInput")
    fblob_d = nc.dram_tensor("fblob", [P, C + 4], F32, kind="ExternalInput")
    out_d = nc.dram_tensor("out", [P, 2, N], BF16, kind="ExternalOutput")
    warm_d = nc.dram_tensor("warm", [P, 1], F32, kind="ExternalOutput")

    with tile.TileContext(nc) as tc:
        with (
            tc.tile_pool(name="consts", bufs=1) as consts,
            tc.tile_pool(name="big", bufs=1) as big,
            tc.tile_pool(name="pA", bufs=1, space="PSUM") as pA,
            tc.tile_pool(name="pB", bufs=1, space="PSUM") as pB,
            tc.tile_pool(name="pS", bufs=2, space="PSUM") as pS,
            tc.tile_pool(name="pU", bufs=1, space="PSUM") as pU,
        ):
            # ---------------- PE warmup under the input DMAs ----------------
            dummy = consts.tile([P, 512], BF16)
            nc.gpsimd.memset(dummy[:], 0)
            warm_sb = consts.tile([P, 1], F32)
            # 1-element exp: forces the implicit ACT_TABLE_LOAD (~1.3us) to
            # run at t~0 under the DMAs
            nc.scalar.activation(warm_sb[0:1, 0:1], dummy[0:1, 0:1], AF.Exp)
            psd = pU.tile([P, 512], F32, tag="u", name="warm")
            for w in range(N_WARM):
                nc.tensor.matmul(psd[:], dummy[:, :P], dummy[:],
                                 start=(w == 0), stop=(w == N_WARM - 1))
            nc.vector.tensor_copy(warm_sb, psd[:, 0:1])
            nc.sync.dma_start(warm_d[:], warm_sb)

            # ---------------- inputs ----------------
            # x pieces ordered to unblock the exp(0) j-pieces in DMA-arrival
            # order: A[0:512], A[512:1280], B[1792:2304], B[1280:1792]
            xt = big.tile([P, 2, N], BF16)
            fblob = consts.tile([P, C + 4], F32)
            wqk = consts.tile([P, 2 * P], BF16)
            wrest = consts.tile([P, 2 * C], BF16)
            x_r = x_d[:].rearrange("c p n -> p c n")
            nc.sync.dma_start(wqk[:], wqk_d[:])
            nc.sync.dma_start(xt[:, :, 0:512], x_r[:, :, 0:512])
            nc.sync.dma_start(fblob, fblob_d[:])
            nc.sync.dma_start(xt[:, :, 512:1280], x_r[:, :, 512:1280])
            nc.sync.dma_start(xt[:, :, 1792:N], x_r[:, :, 1792:N])
            nc.sync.dma_start(xt[:, :, 1280:1792], x_r[:, :, 1280:1792])
            nc.sync.dma_start(wrest[:], wrest_d[:])

            def wv(c):
                return wrest[:, c * C:(c + 1) * C]

            qb = fblob[0:CK, 0:1]
            kb = fblob[0:CK, 1:2]
            vb = fblob[:, 4:C + 4]
            gbias = fblob[:, 2:4]

            # ---------------- shared SBUF tensors ----------------
            q_t = big.tile([CK, N], BF16)
            k_t = big.tile([CK, N], BF16)
            vts = big.tile([P, NI, C], BF16)      # V^T * (Z/s)
            # ones row + bvg row for the rank-1 V-bias matmul term
            ones_r = consts.tile([1, P], BF16)
            nc.gpsimd.memset(ones_r[:], 1.0)
            bvg_r = consts.tile([1, C], BF16)
            nc.vector.tensor_copy(bvg_r[:], fblob[0:1, 4:C + 4])
            v8 = big.tile([P, NP, 2, C], F8)      # fp8 copy of vts, paired
            e8 = big.tile([P, NP, 2, N], F8)      # exp(E), fp8, paired
            ot_all = big.tile([P, 2, N], BF16)    # output staging
            ft1 = big.tile([P, 2, 512], BF16)     # Pool B-half fold
            jnkA = big.tile([P, 2, 640], BF16)    # stt byproducts
            jnkB = big.tile([P, 2, 256], BF16)
            sAB = big.tile([P, NI, 2], F32)       # per-half row sums
            s_sc = big.tile([P, NI], F32)
            sZ = big.tile([P, NI], F32)           # Z / s

            # ---------------- head: Q/K projections + E(0) ----------------
            def psk_mms(j0, jw, eng="dve"):
                psk = pS.tile([P, 512], F32, tag="s", name="psk")
                for c in range(2):
                    nc.tensor.matmul(psk[:CK, :jw],
                                     wqk[:, c * P + CK:(c + 1) * P],
                                     xt[:, c, j0:j0 + jw],
                                     start=(c == 0), stop=(c == 1))
                if eng == "act":
                    nc.scalar.activation(k_t[:, j0:j0 + jw], psk[:CK, :jw],
                                         AF.Identity, bias=kb)
                else:
                    nc.vector.tensor_scalar_add(k_t[:, j0:j0 + jw],
                                                psk[:CK, :jw], kb)

            eps_of = {}

            def alloc_eps(kk):
                epsA = pA.tile([P, EA], F32, tag="A", name=f"epsA{kk}")
                epsB = pB.tile([P, EB], F32, tag="B", name=f"epsB{kk}")
                eps_of[kk] = (epsA, epsB)

            def energy_mms(kk, half, subs):
                (epsA, epsB) = eps_of[kk]
                eps, base = (epsA, 0) if half == 0 else (epsB, EA)
                for (o0, ow) in subs:
                    nc.tensor.matmul(
                        eps[:, o0:o0 + ow],
                        q_t[:, kk * P:(kk + 1) * P],
                        k_t[:, base + o0:base + o0 + ow],
                        start=True, stop=True)

            def emit_exp(kk, half, o0, ow, accum=None):
                (epsA, epsB) = eps_of[kk]
                eps, base = (epsA, 0) if half == 0 else (epsB, EA)
                pr, sl = kk // 2, kk % 2
                nc.scalar.activation(
                    out=e8[:, pr, sl, base + o0:base + o0 + ow],
                    in_=eps[:, o0:o0 + ow], func=AF.Exp, accum_out=accum)

            # PE program order matches dependency readiness (x-piece arrival)
            psk_mms(0, 512)                               # k_t[0:512]
            psq0 = pS.tile([P, 512], F32, tag="s", name="psq0")
            for c in range(2):
                nc.tensor.matmul(psq0[:CK, :P], wqk[:, c * P:c * P + CK],
                                 xt[:, c, 0:P], start=(c == 0), stop=(c == 1))
            nc.scalar.activation(q_t[:, 0:P], psq0[:CK, :P], AF.Identity,
                                 bias=qb)
            alloc_eps(0)
            energy_mms(0, 0, [(0, 512)])
            emit_exp(0, 0, 0, 512)                        # exp(0) A[0:512]
            psk_mms(512, 512, eng="act")                  # x piece 2
            psk_mms(1024, 256, eng="act")
            energy_mms(0, 0, [(512, 512), (1024, 256)])
            emit_exp(0, 0, 512, 768)                      # exp(0) A[512:1280]
            psk_mms(1792, 512)                            # x piece 3
            energy_mms(0, 1, [(512, 512)])
            emit_exp(0, 1, 512, 512)                      # exp(0) B[1792:]
            psk_mms(1280, 512)                            # x piece 4
            energy_mms(0, 1, [(0, 512)])
            emit_exp(0, 1, 0, 512)                        # exp(0) B[1280:1792]

            # q_t j-tiles beyond [0:128]: (j0, jw, due-iteration)
            psq_tiles = [(128, 512, 0), (640, 512, 2), (1152, 512, 5),
                         (1664, 512, 8), (2176, 128, 11)]

            def psq_mms(j0, jw):
                psq = pU.tile([P, 512], F32, tag="u", name="psq")
                for c in range(2):
                    nc.tensor.matmul(psq[:CK, :jw], wqk[:, c * P:c * P + CK],
                                     xt[:, c, j0:j0 + jw],
                                     start=(c == 0), stop=(c == 1))
                nc.vector.tensor_scalar_add(q_t[:, j0:j0 + jw],
                                            psq[:CK, :jw], qb)

            # V-projection: bias folded in as a rank-1 matmul term; the PSUM
            # tile is held until this chunk's sZ lands, then scaled straight
            # into vts (saves the separate bias-add + scale on DVE)
            psv_tiles = {}

            def psv_mms(i):
                psv = pU.tile([P, 512], F32, tag="u", name=f"psv{i}")
                psv_tiles[i] = psv
                for c in range(2):
                    nc.tensor.matmul(psv[:, :C],
                                     xt[:, c, i * P:(i + 1) * P],
                                     wv(c), start=(c == 0), stop=False)
                nc.tensor.matmul(psv[:, :C], ones_r[:], bvg_r[:],
                                 start=False, stop=True)

            # pU work items per pass-1 iteration: one psv per chunk (its PSUM
            # is held until the chunk's vts scale, so psv(k) is emitted in
            # iteration k-1; psq-j1 and psv0 run in the head) + psq tiles on
            # their due iterations
            pu_sched = {it: [] for it in range(NI)}
            psq_next = 1
            for it in range(NI):
                if psq_next < len(psq_tiles) and psq_tiles[psq_next][2] <= it:
                    j0, jw, _ = psq_tiles[psq_next]
                    pu_sched[it].append(("psq", j0, jw))
                    psq_next += 1
                if it >= 1:
                    pu_sched[it].append(("psv", it, 0))

            # head: psq-j1 (gates E(1)A) then psv0 through the pU rotation
            psq_mms(*psq_tiles[0][:2])
            psv_mms(0)

            st_tiles = [None, None]
            st_next_kk = [0, 0]

            def stream_mms(gi, upto_kk):
                (oc, j0, jw) = STREAM_GROUPS[gi]
                while st_next_kk[gi] <= min(upto_kk, NI - 2):
                    kk = st_next_kk[gi]
                    nc.tensor.matmul(
                        st_tiles[gi][:, :jw],
                        vts[:, kk, oc * P:(oc + 1) * P],
                        e8[:, kk // 2, kk % 2, j0:j0 + jw],
                        start=(kk == 0), stop=False)
                    st_next_kk[gi] += 1

            # ---------------- pass-1 loop ----------------
            for k in range(NI):
                pr, sl = k // 2, k % 2
                last = (k == NI - 1)
                if k >= 1:
                    # chunk 17's row sums ride the (now free) ACT accum path
                    emit_exp(k, 0, 0, EA,
                             accum=sAB[:, k, 0:1] if last else None)
                    emit_exp(k, 1, 0, EB,
                             accum=sAB[:, k, 1:2] if last else None)
                a_half = e8[:, pr, sl, 0:EA]
                b_half = e8[:, pr, sl, EA:N]
                if not last:
                    # Pool: fold the B half once; DVE fold-reduces it
                    nc.gpsimd.tensor_tensor(ft1[:, k % 2], b_half[:, 0:512],
                                            b_half[:, 512:1024], ALU.add)
                # psq first (its PE matmul and DVE bias gate the energy
                # chain), then this chunk's psv: its pU slot frees exactly at
                # the previous chunk's vts scale, so it never parks for long
                for item in pu_sched[k]:
                    if item[0] == "psq":
                        psq_mms(item[1], item[2])
                    else:
                        psv_mms(item[1])
                # PE: next chunk's energy + streamed pass-2
                if k + 1 < NI:
                    alloc_eps(k + 1)
                    energy_mms(k + 1, 0, A_SUBS)
                if k >= 1:
                    if k == 1:
                        st_tiles[0] = pS.tile([P, 512], F32, tag="s",
                                              name="st0")
                        st_tiles[1] = pS.tile([P, 512], F32, tag="s",
                                              name="st1")
                    # lag streams 2 chunks behind so vts is always ready and
                    # a parked stream matmul can't block the energy chain
                    stream_mms(0, k - 2)
                    stream_mms(1, k - 2)
                if k + 1 < NI:
                    energy_mms(k + 1, 1, B_SUBS)
                # DVE: fused fold+accum row sums (A direct from fp8, folded B)
                if not last:
                    nc.vector.scalar_tensor_tensor(
                        jnkA[:, k % 2], a_half[:, 0:640], 0.0,
                        a_half[:, 640:1280], ALU.add, ALU.add,
                        accum_out=sAB[:, k, 0:1])
                    nc.vector.scalar_tensor_tensor(
                        jnkB[:, k % 2], ft1[:, k % 2, 0:256], 0.0,
                        ft1[:, k % 2, 256:512], ALU.add, ALU.add,
                        accum_out=sAB[:, k, 1:2])
                nc.vector.tensor_tensor(s_sc[:, k:k + 1], sAB[:, k, 0:1],
                                        sAB[:, k, 1:2], ALU.add)
                nc.vector.tensor_scalar_mul(s_sc[:, k:k + 1],
                                            s_sc[:, k:k + 1], 1.0 / Z)
                nc.vector.reciprocal(sZ[:, k:k + 1], s_sc[:, k:k + 1])
                nc.vector.tensor_scalar_mul(vts[:, k], psv_tiles[k][:, :C],
                                            sZ[:, k:k + 1])
                # Pool: fp8 copy of vts for the DoubleRow tail matmuls
                nc.gpsimd.tensor_copy(v8[:, pr, sl, :], vts[:, k])

            # ---------------- pass 2 tail ----------------
            def emit_bias(oc, j0, jw, psum_ap, eng):
                dst = ot_all[:, oc, j0:j0 + jw]
                if eng == "act":
                    nc.scalar.activation(dst, psum_ap, AF.Identity,
                                         bias=gbias[:, oc:oc + 1],
                                         scale=1.0 / Z)
                else:
                    nc.vector.tensor_scalar(dst, psum_ap, 1.0 / Z,
                                            gbias[:, oc:oc + 1],
                                            ALU.mult, ALU.add)

            def tail_p8(oc, j0, jw, ap):
                nc.tensor.matmul(
                    ap, v8[:, NP - 1, :, oc * P:(oc + 1) * P],
                    e8[:, NP - 1, :, j0:j0 + jw],
                    start=False, stop=True, perf_mode=DR)

            # stream closeouts: park early in the PE queue so they run the
            # moment vts[17] lands, then store right away
            st_eng = ["act", "dve"]
            for gi, (oc, j0, jw) in enumerate(STREAM_GROUPS):
                stream_mms(gi, NI - 2)
                kk = NI - 1
                nc.tensor.matmul(
                    st_tiles[gi][:, :jw],
                    vts[:, kk, oc * P:(oc + 1) * P],
                    e8[:, kk // 2, kk % 2, j0:j0 + jw],
                    start=False, stop=True)
                emit_bias(oc, j0, jw, st_tiles[gi][:, :jw], st_eng[gi])
            # tailU: pU frees right after psv17's scale, well before pA/pB
            (ocu, j0u, jwu) = TAIL_U[0]
            tailU_t = pU.tile([P, 512], F32, tag="u", name="tailU")
            for p in range(NP - 1):
                nc.tensor.matmul(
                    tailU_t[:, :jwu], v8[:, p, :, ocu * P:(ocu + 1) * P],
                    e8[:, p, :, j0u:j0u + jwu],
                    start=(p == 0), stop=False, perf_mode=DR)
            tail_p8(ocu, j0u, jwu, tailU_t[:, :jwu])
            emit_bias(ocu, j0u, jwu, tailU_t[:, :jwu], "act")
            nc.sync.dma_start(out_d[:, 0, 0:512], ot_all[:, 0, 0:512])
            nc.sync.dma_start(out_d[:, 1, 0:512], ot_all[:, 1, 0:512])
            nc.sync.dma_start(out_d[:, 0, 1536:2048], ot_all[:, 0, 1536:2048])

            # A/B tail groups: pair-major so all groups complete together
            tailA = pA.tile([P, EA], F32, tag="A", name="tailA")
            tailB = pB.tile([P, EB], F32, tag="B", name="tailB")
            ab_groups = (
                [(oc, j0, jw, tailA[:, o0:o0 + jw]) for (oc, j0, jw, o0)
                 in TAIL_A] +
                [(oc, j0, jw, tailB[:, o0:o0 + jw]) for (oc, j0, jw, o0)
                 in TAIL_B])
            for p in range(NP - 1):
                for (oc, j0, jw, ap) in ab_groups:
                    nc.tensor.matmul(
                        ap, v8[:, p, :, oc * P:(oc + 1) * P],
                        e8[:, p, :, j0:j0 + jw],
                        start=(p == 0), stop=False,
                        perf_mode=DR)
            # p8 + bias + store in store-piece priority order
            ab_by_prio = [ab_groups[0], ab_groups[3], ab_groups[1],
                          ab_groups[4], ab_groups[2]]
            engs = ["dve", "act", "dve", "act", "dve"]
            for (oc, j0, jw, ap), eng in zip(ab_by_prio, engs):
                tail_p8(oc, j0, jw, ap)
                emit_bias(oc, j0, jw, ap, eng)
            nc.sync.dma_start(out_d[:, 0, 512:1536], ot_all[:, 0, 512:1536])
            nc.sync.dma_start(out_d[:, 1, 512:1536], ot_all[:, 1, 512:1536])
            nc.sync.dma_start(out_d[:, 0, 2048:N], ot_all[:, 0, 2048:N])

            # pS-hosted groups (banks free only after the stream closeouts)
            tailS = [pS.tile([P, 512], F32, tag="s", name=f"tailS{i}")
                     for i in range(2)]
            for i, (oc, j0, jw) in enumerate(TAIL_S):
                ap = tailS[i][:, :jw]
                for p in range(NP):
                    nc.tensor.matmul(
                        ap, v8[:, p, :, oc * P:(oc + 1) * P],
                        e8[:, p, :, j0:j0 + jw],
                        start=(p == 0), stop=(p == NP - 1),
                        perf_mode=DR)
                emit_bias(oc, j0, jw, ap, "act" if i else "dve")
            nc.sync.dma_start(out_d[:, 1, 1536:N], ot_all[:, 1, 1536:N])

    nc.compile()
    return nc


_NC_CACHE = []


def _get_nc():
    if not _NC_CACHE:
        _NC_CACHE.append(_build_nc())
    return _NC_CACHE[0]


def _prep_inputs(x, query_weight, query_bias, key_weight, key_bias,
                 value_weight, value_bias, gamma_weight, gamma_bias):
    bf16 = ml_dtypes.bfloat16
    x = np.asarray(x, np.float32).reshape(B, C, N)
    qw = np.asarray(query_weight, np.float32)[:, :, 0, 0]   # (64, 256)
    kw = np.asarray(key_weight, np.float32)[:, :, 0, 0]     # (64, 256)
    vw = np.asarray(value_weight, np.float32)[:, :, 0, 0]   # (256, 256)
    gw = np.asarray(gamma_weight, np.float32)[:, :, 0, 0]   # (256, 256)

    # wqk[p, c*128+m] = W_cat^T[c*128+p, m]  (W_cat = [Wq; Wk], (128, 256))
    wcat_t = np.concatenate([qw, kw], axis=0).T              # (256, 128)
    wqk = np.ascontiguousarray(
        wcat_t.reshape(2, P, P).transpose(1, 0, 2).reshape(P, 2 * P))

    # gamma 1x1 conv folded into the V projection:
    # W_comb = (Wg @ Wv)^T, bvg = Wg @ bv
    w_comb = (gw @ vw).T                                    # (c_in, o)
    wrest = np.ascontiguousarray(
        w_comb.reshape(2, P, C).transpose(1, 0, 2).reshape(P, 2 * C))
    bvg = gw @ np.asarray(value_bias, np.float32)

    fblob = np.zeros((P, C + 4), np.float32)
    fblob[0:CK, 0] = np.asarray(query_bias, np.float32)
    fblob[0:CK, 1] = np.asarray(key_bias, np.float32)
    fblob[:, 2:4] = np.asarray(gamma_bias, np.float32).reshape(2, P).T
    fblob[:, 4:C + 4] = bvg[None, :]

    base = {
        "wqk": wqk.astype(bf16),
        "wrest": wrest.astype(bf16),
        "fblob": fblob,
    }
    in_maps = []
    for b in range(B):
        m = dict(base)
        m["x"] = x[b].reshape(2, P, N).astype(bf16)
        in_maps.append(m)
    return in_maps


def kernel(x, query_weight, query_bias, key_weight, key_bias,
           value_weight, value_bias, gamma_weight, gamma_bias, k):
    assert int(k) == C // CK
    in_maps = _prep_inputs(x, query_weight, query_bias, key_weight, key_bias,
                           value_weight, value_bias, gamma_weight, gamma_bias)
    nc = _get_nc()
    res = run_bass_kernel_spmd(nc, in_maps, core_ids=list(range(NCORES)))

    out = np.empty((B, C, H, W), np.float32)
    for b in range(B):
        # out staged as [P, 2, N] bf16 -> (C, H, W) f32
        ob = np.asarray(res.results[b]["out"], dtype=np.float32)
        out[b] = ob.transpose(1, 0, 2).reshape(C, H, W)
    return out
